# revision 23
# baseline (speedup 1.0000x reference)
"""CrossAttentionBlock Trainium2 kernel.

Math (reference):
    q = Wq@xq + bq        [RC=16, N]     (per-voxel 1x1x1 conv == channel matmul)
    k = Wk@xkv + bk       [16, N]
    v = Wv@xkv + bv       [C=128, N]
    S = (q^T k) / 4       [N, N]
    P = softmax_rows(S)
    out = v @ P^T         [C, N]
    y = x_q + gamma*out

Kernel strategy (8 NeuronCores, sequence-parallel over the N=13824 query
tokens; each core owns NQ=1728 queries against full K/V):
  * Host folds: 1/sqrt(RC) into Wq/bq; gamma into Wv; gamma*bv + x_q into the
    residual (softmax rows sum to 1 so the v-bias is a per-channel constant).
  * Scores are built TRANSPOSED (S^T tiles [128 keys x 432 queries]): k-tile
    stationary, q moving - no transposes anywhere.  Softmax needs no max
    subtraction (|S|<~3 by construction) and normalization is deferred:
    exp(S^T) feeds two accumulating matmuls - out_u = (gamma*v)^T-contracted
    output and a ones-row matmul giving row sums - and the divide happens once
    at the end via reciprocal + a 1->128 partition-broadcast matmul.
  * All three inner matmuls (S^T, out_u, rowsum) run fp8e4 + DoubleRow (2
    MACs/cell/cycle).  K/Q live in the DoubleRow layout [Ki=8, Ko=2, *]
    (virtual row r = p + 8o, staged via an SBUF->SBUF DMA partition remap);
    the out_u/rowsum moving operand pairs two consecutive key tiles.
  * exp is the throughput limit (191M elements through 1-elem/cycle/lane
    engines), so it is split ~53/47 between ScalarE (true exp, fp8 out) and
    VectorE (Schraudolph int8 bit-trick writing e4m3 bit patterns).  To
    amortize each engine's fixed per-op cost, S^T tiles live in a manual
    6-slot single-PSUM-bank arena and exp runs on 3 slots at a time with a
    single strided access pattern, writing a 12-slot SBUF fp8 ring that the
    matmuls consume in pairs.  Attention contributes O(1e-4) of the output
    magnitude, so ~6% fp8 quantization is invisible; the residual is fp32.
"""

import contextlib

import numpy as np
import ml_dtypes

import concourse.bass as bass
import concourse.mybir as mybir
from concourse import bacc
from concourse.tile import TileContext
from concourse.bass_utils import run_bass_kernel_spmd

F32 = mybir.dt.float32
BF16 = mybir.dt.bfloat16
FP8 = mybir.dt.float8e4
I8 = mybir.dt.int8
AF = mybir.ActivationFunctionType
DR = mybir.MatmulPerfMode.DoubleRow

C = 128           # channels
RC = 16           # reduced (q/k) channels
D = H = W = 24
N = D * H * W     # 13824 tokens
NCORES = 8
NQ = N // NCORES  # 1728 queries per core
CHUNK = 432       # query chunk ([128, CHUNK] fp32 fits one PSUM bank)
NCHUNKS = NQ // CHUNK   # 4
MT = N // 128     # 108 key tiles of 128
PAIRS = MT // 2   # 54 key-tile pairs per chunk
LAGP = 4          # out/rs matmuls trail exp by this many pairs (PE is in-order;
                  # the lag must cover exp latency with PE work or PE stalls)

LOG2E = 1.4426950408889634
EXP8_SCALE = 8.0 * LOG2E      # e4m3: 3 mantissa bits, bias 7
EXP8_BIAS = 56.0 - 0.3        # 7*8 + Schraudolph offset
# exp pair -> engine: Bresenham-interleaved so ScalarE/VectorE overlap
# (runs of the same engine would serialize the whole pipeline)
ACT_FRAC = 0.53


def _act_pattern(n):
    pat, acc = [], 0.0
    for _ in range(n):
        acc += ACT_FRAC
        if acc >= 1.0:
            acc -= 1.0
            pat.append(True)
        else:
            pat.append(False)
    return pat

_BUILD_CACHE: dict = {}


def build_nc(repeats: int = 1):
    """Build + compile the per-core Bass program (SPMD across 8 cores)."""
    key = repeats
    if key in _BUILD_CACHE:
        return _BUILD_CACHE[key]

    nc = bacc.Bacc("TRN2", target_bir_lowering=False, debug=False,
                   num_devices=NCORES)
    xq = nc.dram_tensor("xq", [C, NQ], F32, kind="ExternalInput").ap()
    xkv = nc.dram_tensor("xkv", [C, N], BF16, kind="ExternalInput").ap()
    wqT = nc.dram_tensor("wqT", [C, RC], BF16, kind="ExternalInput").ap()
    wkT = nc.dram_tensor("wkT", [C, RC], BF16, kind="ExternalInput").ap()
    wvT = nc.dram_tensor("wvT", [C, C], BF16, kind="ExternalInput").ap()
    bq = nc.dram_tensor("bq", [RC, 1], F32, kind="ExternalInput").ap()
    bk = nc.dram_tensor("bk", [RC, 1], F32, kind="ExternalInput").ap()
    y = nc.dram_tensor("y", [C, NQ], F32, kind="ExternalOutput").ap()

    with TileContext(nc) as tc, contextlib.ExitStack() as ctx:
        cpool = ctx.enter_context(tc.tile_pool(name="consts", bufs=1))
        ppool = ctx.enter_context(tc.tile_pool(name="psum", bufs=1, space="PSUM"))
        spool = ctx.enter_context(tc.tile_pool(name="work", bufs=1))

        # ---- resident inputs -------------------------------------------------
        xq_sb = cpool.tile([C, NQ], F32)
        nc.sync.dma_start(xq_sb[:], xq[:])
        xkv_sb = cpool.tile([C, N], BF16)
        nc.sync.dma_start(xkv_sb[:], xkv[:])
        wqT_sb = cpool.tile([C, RC], BF16)
        nc.sync.dma_start(wqT_sb[:], wqT[:])
        wkT_sb = cpool.tile([C, RC], BF16)
        nc.sync.dma_start(wkT_sb[:], wkT[:])
        wvT_sb = cpool.tile([C, C], BF16)
        nc.sync.dma_start(wvT_sb[:], wvT[:])
        bq_sb = cpool.tile([RC, 1], F32)
        nc.sync.dma_start(bq_sb[:], bq[:])
        bk_sb = cpool.tile([RC, 1], F32)
        nc.sync.dma_start(bk_sb[:], bk[:])

        # lhsT for DoubleRow row-sum matmul; padded so the Ko step is 16B
        # (ISA requires step%16==0 on the DoubleRow stationary AP)
        ones_db = cpool.tile([C, 32], FP8)
        nc.gpsimd.memset(ones_db[:], 1.0)
        ones_row = cpool.tile([1, C], BF16)   # lhsT for 1->128 broadcast matmul
        nc.gpsimd.memset(ones_row[:], 1.0)

        # ---- projections -----------------------------------------------------
        xq_bf = cpool.tile([C, NQ], BF16)
        nc.gpsimd.tensor_copy(xq_bf[:], xq_sb[:])

        # prologue psum traffic shares the S^T pair-supertile slots (tag "st")
        def slot_ap(parts, width):
            t = ppool.tile([C, 1024], F32, tag="st", bufs=3, name="pslot")
            return t[0:parts, 0:width]

        k_tmp = cpool.tile([RC, N], FP8)
        for i in range(N // 512):
            sl = bass.ts(i, 512)
            psk = slot_ap(RC, 512)
            nc.tensor.matmul(psk, wkT_sb[:], xkv_sb[:, sl], start=True, stop=True)
            if i % 2 == 0:
                nc.scalar.activation(k_tmp[:, sl], psk, AF.Identity, bias=bk_sb[:])
            else:
                nc.vector.tensor_scalar(out=k_tmp[:, sl], in0=psk,
                                        scalar1=bk_sb[:], scalar2=None,
                                        op0=mybir.AluOpType.add)

        q_tmp = cpool.tile([RC, NQ], FP8)
        for ch in range(NCHUNKS):
            sl = bass.ts(ch, CHUNK)
            psq = slot_ap(RC, CHUNK)
            nc.tensor.matmul(psq, wqT_sb[:], xq_bf[:, sl], start=True, stop=True)
            nc.scalar.activation(q_tmp[:, sl], psq, AF.Identity, bias=bq_sb[:])

        # DoubleRow layout [8, 2, *]: virtual row r = p + 8*o.  k_db DMAs are
        # split so early key tiles unlock before the whole projection lands.
        QN = N // 4
        k_db = cpool.tile([8, 2 * N], FP8)
        for qq in range(4):
            lo, hi = qq * QN, (qq + 1) * QN
            nc.sync.dma_start(k_db[:, lo:hi], k_tmp[0:8, lo:hi])
            nc.sync.dma_start(k_db[:, N + lo:N + hi], k_tmp[8:16, lo:hi])
        q_db = cpool.tile([8, 2 * NQ], FP8)
        nc.sync.dma_start(q_db[:, 0:NQ], q_tmp[0:8, :])
        nc.sync.dma_start(q_db[:, NQ:2 * NQ], q_tmp[8:16, :])
        q3 = q_db.rearrange("p (o x) -> p o x", o=2)
        k3 = k_db.rearrange("p (o x) -> p o x", o=2)

        # v^T tiles (tile t: [m_local(128), c] = gamma*v[c, 128t+m]), evacuated
        # from PSUM four tiles per op to amortize the fixed engine cost.
        vt_sb = cpool.tile([C, N], FP8)
        for qd in range(MT // 4):
            psv = slot_ap(C, 512)
            for j in range(4):
                t = 4 * qd + j
                nc.tensor.matmul(psv[:, bass.ts(j, 128)], xkv_sb[:, bass.ts(t, 128)],
                                 wvT_sb[:], start=True, stop=True)
            dst = vt_sb[:, bass.ts(qd, 512)]
            if qd % 2 == 0:
                nc.scalar.copy(dst, psv[:])
            else:
                nc.vector.tensor_copy(dst, psv[:])

        # ---- attention main loop --------------------------------------------
        act_pat = _act_pattern(NCHUNKS * PAIRS * max(repeats, 1))
        for rep in range(repeats):
            for ch in range(NCHUNKS):
                sl = bass.ts(ch, CHUNK)
                outu = ppool.tile([C, CHUNK], F32, tag="outu")
                rs = ppool.tile([1, CHUNK], F32, tag="rs")
                gidx = (rep * NCHUNKS + ch) * PAIRS
                ex_tiles = {}
                for up in range(PAIRS + LAGP):
                    if up < PAIRS:
                        s = up
                        stp = ppool.tile([C, 1024], F32, tag="st", bufs=3)
                        for j in range(2):
                            t = 2 * s + j
                            nc.tensor.matmul(stp[:, 512 * j:512 * j + CHUNK],
                                             k3[:, :, bass.ts(t, 128)],
                                             q3[:, :, sl],
                                             start=True, stop=True, perf_mode=DR)
                        st3 = stp.rearrange("p (b x) -> p b x", b=2)[:, :, 0:CHUNK]
                        ex = spool.tile([C, 2 * CHUNK], FP8, tag="ex", bufs=LAGP + 3)
                        ex3 = ex.rearrange("p (b x) -> p b x", b=2)
                        if act_pat[gidx + s]:
                            nc.scalar.activation(ex3, st3, AF.Exp)
                        else:
                            nc.vector.tensor_scalar(
                                out=ex3.bitcast(I8), in0=st3,
                                scalar1=EXP8_SCALE, scalar2=EXP8_BIAS,
                                op0=mybir.AluOpType.mult,
                                op1=mybir.AluOpType.add)
                        ex_tiles[s] = ex
                    if up >= LAGP:
                        s = up - LAGP
                        ex = ex_tiles.pop(s)
                        ex3 = ex.rearrange("p (b x) -> p b x", b=2)
                        vt3 = vt_sb[:, bass.ds(256 * s, 256)].rearrange(
                            "p (b c) -> p b c", b=2)
                        nc.tensor.matmul(outu[:], vt3, ex3, perf_mode=DR,
                                         start=(s == 0), stop=(s == PAIRS - 1))
                        o3 = ones_db.rearrange("p (b c) -> p b c", b=2)[:, :, 0:1]
                        nc.tensor.matmul(rs[:], o3, ex3, perf_mode=DR,
                                         start=(s == 0), stop=(s == PAIRS - 1))
                # normalize + residual.  outu is evacuated immediately so the
                # next chunk's accumulation isn't blocked by this chain; the
                # elementwise tail runs on the otherwise-idle GpSimd engine.
                outu_s = spool.tile([C, CHUNK], F32, tag="outu_s", bufs=2)
                nc.scalar.copy(outu_s[:], outu[:])
                recip = spool.tile([1, CHUNK], F32, tag="recip", bufs=2)
                nc.vector.reciprocal_approx_fast(out=recip[:], in_=rs[:])
                recip_bf = spool.tile([1, CHUNK], BF16, tag="recipb", bufs=2)
                nc.gpsimd.tensor_copy(recip_bf[:], recip[:])
                bcp = slot_ap(C, CHUNK)
                nc.tensor.matmul(bcp, ones_row[:], recip_bf[:], start=True, stop=True)
                bcs = spool.tile([C, CHUNK], F32, tag="bcs", bufs=2)
                nc.scalar.copy(bcs[:], bcp)
                t1 = spool.tile([C, CHUNK], F32, tag="t1", bufs=2)
                nc.gpsimd.tensor_mul(t1[:], outu_s[:], bcs[:])
                res = spool.tile([C, CHUNK], F32, tag="res", bufs=2)
                nc.gpsimd.tensor_add(res[:], t1[:], xq_sb[:, sl])
                nc.sync.dma_start(y[:, sl], res[:])
            if rep != repeats - 1:
                tc.strict_bb_all_engine_barrier()

    nc.compile()
    _BUILD_CACHE[key] = nc
    return nc


def _prep_in_maps(x_q, x_kv, Wq, bq, Wk, bk, Wv, bv, gamma):
    bf16 = ml_dtypes.bfloat16
    f32 = np.float32
    x_q = np.asarray(x_q, f32).reshape(C, N)
    x_kv = np.asarray(x_kv, f32).reshape(C, N)
    Wq = np.asarray(Wq, f32)
    bq = np.asarray(bq, f32)
    Wk = np.asarray(Wk, f32)
    bk = np.asarray(bk, f32)
    Wv = np.asarray(Wv, f32)
    bv = np.asarray(bv, f32)
    gamma = float(np.asarray(gamma, f32).reshape(()))

    scale = 1.0 / np.sqrt(np.float32(RC))
    xkv_b = np.ascontiguousarray(x_kv).astype(bf16)
    wqT = np.ascontiguousarray(Wq.T * scale).astype(bf16)
    wkT = np.ascontiguousarray(Wk.T).astype(bf16)
    wvT = np.ascontiguousarray(Wv.T * gamma).astype(bf16)
    bq_s = np.ascontiguousarray((bq * scale).reshape(RC, 1))
    bk_s = np.ascontiguousarray(bk.reshape(RC, 1))
    resid_bias = (gamma * bv).astype(f32)  # softmax rows sum to 1

    in_maps = []
    for c in range(NCORES):
        xq_slice = np.ascontiguousarray(
            x_q[:, c * NQ:(c + 1) * NQ] + resid_bias[:, None], f32)
        in_maps.append({
            "xq": xq_slice, "xkv": xkv_b,
            "wqT": wqT, "wkT": wkT, "wvT": wvT,
            "bq": bq_s, "bk": bk_s,
        })
    return in_maps


def kernel(x_q, x_kv, Wq, bq, Wk, bk, Wv, bv, gamma):
    nc = build_nc(repeats=1)
    in_maps = _prep_in_maps(x_q, x_kv, Wq, bq, Wk, bk, Wv, bv, gamma)
    res = run_bass_kernel_spmd(nc, in_maps, list(range(NCORES)))
    out = np.concatenate([res.results[c]["y"] for c in range(NCORES)], axis=1)
    return out.reshape(1, C, D, H, W).astype(np.float32)


# revision 29
# speedup vs baseline: 1.0589x; 1.0589x over previous
"""CrossAttentionBlock Trainium2 kernel.

Math (reference):
    q = Wq@xq + bq        [RC=16, N]     (per-voxel 1x1x1 conv == channel matmul)
    k = Wk@xkv + bk       [16, N]
    v = Wv@xkv + bv       [C=128, N]
    S = (q^T k) / 4       [N, N]
    P = softmax_rows(S)
    out = v @ P^T         [C, N]
    y = x_q + gamma*out

Kernel strategy (8 NeuronCores, sequence-parallel over the N=13824 query
tokens; each core owns NQ=1728 queries against full K/V):
  * Host folds: 1/sqrt(RC) into Wq/bq; gamma into Wv; gamma*bv + x_q into the
    residual (softmax rows sum to 1 so the v-bias is a per-channel constant).
  * Scores are built TRANSPOSED (S^T tiles [128 keys x 432 queries]): k-tile
    stationary, q moving - no transposes anywhere.  Softmax needs no max
    subtraction (|S|<~3 by construction) and normalization is deferred:
    exp(S^T) feeds two accumulating matmuls - out_u = (gamma*v)^T-contracted
    output and a ones-row matmul giving row sums - and the divide happens once
    at the end via reciprocal + a 1->128 partition-broadcast matmul.
  * All three inner matmuls (S^T, out_u, rowsum) run fp8e4 + DoubleRow (2
    MACs/cell/cycle).  K/Q live in the DoubleRow layout [Ki=8, Ko=2, *]
    (virtual row r = p + 8o, staged via an SBUF->SBUF DMA partition remap);
    the out_u/rowsum moving operand pairs two consecutive key tiles.
  * exp is the throughput limit (191M elements through 1-elem/cycle/lane
    engines), so it is split ~53/47 between ScalarE (true exp, fp8 out) and
    VectorE (Schraudolph int8 bit-trick writing e4m3 bit patterns).  To
    amortize each engine's fixed per-op cost, S^T tiles live in a manual
    6-slot single-PSUM-bank arena and exp runs on 3 slots at a time with a
    single strided access pattern, writing a 12-slot SBUF fp8 ring that the
    matmuls consume in pairs.  Attention contributes O(1e-4) of the output
    magnitude, so ~6% fp8 quantization is invisible; the residual is fp32.
"""

import contextlib

import numpy as np
import ml_dtypes

import concourse.bass as bass
import concourse.mybir as mybir
from concourse import bacc
from concourse.tile import TileContext
from concourse.bass_utils import run_bass_kernel_spmd

F32 = mybir.dt.float32
BF16 = mybir.dt.bfloat16
FP8 = mybir.dt.float8e4
I8 = mybir.dt.int8
AF = mybir.ActivationFunctionType
DR = mybir.MatmulPerfMode.DoubleRow

C = 128           # channels
RC = 16           # reduced (q/k) channels
D = H = W = 24
N = D * H * W     # 13824 tokens
NCORES = 8
NQ = N // NCORES  # 1728 queries per core
CHUNK = 432       # query chunk ([128, CHUNK] fp32 fits one PSUM bank)
NCHUNKS = NQ // CHUNK   # 4
MT = N // 128     # 108 key tiles of 128
PAIRS = MT // 2   # 54 key-tile pairs per chunk
LAGP = 6          # out/rs matmuls trail exp by this many pairs (PE is in-order;
                  # the lag must cover exp latency with PE work or PE stalls)

LOG2E = 1.4426950408889634
EXP8_SCALE = 8.0 * LOG2E      # e4m3: 3 mantissa bits, bias 7
EXP8_BIAS = 56.0 - 0.3        # 7*8 + Schraudolph offset
# exp pair -> engine: Bresenham-interleaved so ScalarE/VectorE overlap
# (runs of the same engine would serialize the whole pipeline)
ACT_FRAC = 0.53


def _act_pattern(n):
    pat, acc = [], 0.0
    for _ in range(n):
        acc += ACT_FRAC
        if acc >= 1.0:
            acc -= 1.0
            pat.append(True)
        else:
            pat.append(False)
    return pat

_BUILD_CACHE: dict = {}


def build_nc(repeats: int = 1):
    """Build + compile the per-core Bass program (SPMD across 8 cores)."""
    key = repeats
    if key in _BUILD_CACHE:
        return _BUILD_CACHE[key]

    nc = bacc.Bacc("TRN2", target_bir_lowering=False, debug=False,
                   num_devices=NCORES)
    xq = nc.dram_tensor("xq", [C, NQ], F32, kind="ExternalInput").ap()
    xkv = nc.dram_tensor("xkv", [C, N], BF16, kind="ExternalInput").ap()
    wqT = nc.dram_tensor("wqT", [C, RC], BF16, kind="ExternalInput").ap()
    wkT = nc.dram_tensor("wkT", [C, RC], BF16, kind="ExternalInput").ap()
    wvT = nc.dram_tensor("wvT", [C, C], BF16, kind="ExternalInput").ap()
    bq = nc.dram_tensor("bq", [RC, 1], F32, kind="ExternalInput").ap()
    bk = nc.dram_tensor("bk", [RC, 1], F32, kind="ExternalInput").ap()
    y = nc.dram_tensor("y", [C, NQ], F32, kind="ExternalOutput").ap()

    with TileContext(nc) as tc, contextlib.ExitStack() as ctx:
        cpool = ctx.enter_context(tc.tile_pool(name="consts", bufs=1))
        ppool = ctx.enter_context(tc.tile_pool(name="psum", bufs=1, space="PSUM"))
        spool = ctx.enter_context(tc.tile_pool(name="work", bufs=1))

        # ---- resident inputs -------------------------------------------------
        xq_sb = cpool.tile([C, NQ], F32)
        nc.sync.dma_start(xq_sb[:], xq[:])
        xkv_sb = cpool.tile([C, N], BF16)
        nc.sync.dma_start(xkv_sb[:], xkv[:])
        wqT_sb = cpool.tile([C, RC], BF16)
        nc.sync.dma_start(wqT_sb[:], wqT[:])
        wkT_sb = cpool.tile([C, RC], BF16)
        nc.sync.dma_start(wkT_sb[:], wkT[:])
        wvT_sb = cpool.tile([C, C], BF16)
        nc.sync.dma_start(wvT_sb[:], wvT[:])
        bq_sb = cpool.tile([RC, 1], F32)
        nc.sync.dma_start(bq_sb[:], bq[:])
        bk_sb = cpool.tile([RC, 1], F32)
        nc.sync.dma_start(bk_sb[:], bk[:])

        # lhsT for DoubleRow row-sum matmul; padded so the Ko step is 16B
        # (ISA requires step%16==0 on the DoubleRow stationary AP)
        ones_db = cpool.tile([C, 32], FP8)
        nc.gpsimd.memset(ones_db[:], 1.0)
        ones_row = cpool.tile([1, C], BF16)   # lhsT for 1->128 broadcast matmul
        nc.gpsimd.memset(ones_row[:], 1.0)

        # ---- projections -----------------------------------------------------
        xq_bf = cpool.tile([C, NQ], BF16)
        nc.gpsimd.tensor_copy(xq_bf[:], xq_sb[:])

        # Prologue psum traffic rotates through the S^T pair-supertile slots
        # AND the (not-yet-live) outu/rs bank slots - 5 banks of pipelining
        # for the projection evacuations instead of 3.
        _pcnt = [0]

        def slot_ap(parts, width):
            i = _pcnt[0] % 5
            _pcnt[0] += 1
            if i < 3:
                t = ppool.tile([C, 1024], F32, tag="st", bufs=3, name="pslot")
            elif i == 3:
                t = ppool.tile([C, 512], F32, tag="outu", bufs=1, name="pslot_o")
            else:
                t = ppool.tile([C, 512], F32, tag="rs", bufs=1, name="pslot_r")
            return t[0:parts, 0:width]

        k_tmp = cpool.tile([RC, N], FP8)
        for i in range(N // 512):
            sl = bass.ts(i, 512)
            psk = slot_ap(RC, 512)
            nc.tensor.matmul(psk, wkT_sb[:], xkv_sb[:, sl], start=True, stop=True)
            if i % 2 == 0:
                nc.scalar.activation(k_tmp[:, sl], psk, AF.Identity, bias=bk_sb[:])
            else:
                nc.vector.tensor_scalar(out=k_tmp[:, sl], in0=psk,
                                        scalar1=bk_sb[:], scalar2=None,
                                        op0=mybir.AluOpType.add)

        q_tmp = cpool.tile([RC, NQ], FP8)
        for ch in range(NCHUNKS):
            sl = bass.ts(ch, CHUNK)
            psq = slot_ap(RC, CHUNK)
            nc.tensor.matmul(psq, wqT_sb[:], xq_bf[:, sl], start=True, stop=True)
            nc.scalar.activation(q_tmp[:, sl], psq, AF.Identity, bias=bq_sb[:])

        # DoubleRow layout [8, 2, *]: virtual row r = p + 8*o.  k_db DMAs are
        # split so early key tiles unlock before the whole projection lands.
        QN = N // 4
        k_db = cpool.tile([8, 2 * N], FP8)
        for qq in range(4):
            lo, hi = qq * QN, (qq + 1) * QN
            nc.sync.dma_start(k_db[:, lo:hi], k_tmp[0:8, lo:hi])
            nc.sync.dma_start(k_db[:, N + lo:N + hi], k_tmp[8:16, lo:hi])
        q_db = cpool.tile([8, 2 * NQ], FP8)
        nc.sync.dma_start(q_db[:, 0:NQ], q_tmp[0:8, :])
        nc.sync.dma_start(q_db[:, NQ:2 * NQ], q_tmp[8:16, :])
        q3 = q_db.rearrange("p (o x) -> p o x", o=2)
        k3 = k_db.rearrange("p (o x) -> p o x", o=2)

        # v^T tiles (tile t: [m_local(128), c] = gamma*v[c, 128t+m]), evacuated
        # from PSUM four tiles per op to amortize the fixed engine cost.
        vt_sb = cpool.tile([C, N], FP8)
        for qd in range(MT // 4):
            psv = slot_ap(C, 512)
            for j in range(4):
                t = 4 * qd + j
                nc.tensor.matmul(psv[:, bass.ts(j, 128)], xkv_sb[:, bass.ts(t, 128)],
                                 wvT_sb[:], start=True, stop=True)
            dst = vt_sb[:, bass.ts(qd, 512)]
            if qd % 2 == 0:
                nc.scalar.copy(dst, psv[:])
            else:
                nc.vector.tensor_copy(dst, psv[:])

        # ---- attention main loop --------------------------------------------
        # The per-chunk normalize+residual epilogue is deferred into the NEXT
        # chunk's pipeline (two stages) so its PE/ACT ops never head-of-line
        # block the steady-state stream.
        act_pat = _act_pattern(NCHUNKS * PAIRS * max(repeats, 1))
        pend = {}

        def epi_a():
            # free outu/rs as early as possible
            pend["outu_s"] = outu_s = spool.tile([C, CHUNK], F32, name="outu_s",
                                                 tag="outu_s", bufs=2)
            nc.scalar.copy(outu_s[:], pend.pop("outu")[:])
            recip = spool.tile([1, CHUNK], F32, tag="recip", bufs=2)
            nc.vector.reciprocal_approx_fast(out=recip[:], in_=pend.pop("rs")[:])
            pend["recip_bf"] = recip_bf = spool.tile([1, CHUNK], BF16,
                                                     name="recip_bf",
                                                     tag="recipb", bufs=2)
            nc.gpsimd.tensor_copy(recip_bf[:], recip[:])

        def epi_b():
            sl = pend.pop("sl")
            bcpt = ppool.tile([C, 1024], F32, tag="st", bufs=3, name="bcpt")
            bcp = bcpt[:, 0:CHUNK]
            nc.tensor.matmul(bcp, ones_row[:], pend.pop("recip_bf")[:],
                             start=True, stop=True)
            bcs = spool.tile([C, CHUNK], F32, tag="bcs", bufs=2)
            nc.scalar.copy(bcs[:], bcp)
            t1 = spool.tile([C, CHUNK], F32, tag="t1", bufs=2)
            nc.gpsimd.tensor_mul(t1[:], pend.pop("outu_s")[:], bcs[:])
            res = spool.tile([C, CHUNK], F32, tag="res", bufs=2)
            nc.gpsimd.tensor_add(res[:], t1[:], xq_sb[:, sl])
            nc.sync.dma_start(y[:, sl], res[:])

        for rep in range(repeats):
            for ch in range(NCHUNKS):
                sl = bass.ts(ch, CHUNK)
                outu = ppool.tile([C, CHUNK], F32, tag="outu")
                rs = ppool.tile([1, CHUNK], F32, tag="rs")
                gidx = (rep * NCHUNKS + ch) * PAIRS
                ex_tiles = {}
                for up in range(PAIRS + LAGP):
                    if up == 1 and "outu" in pend:
                        epi_a()
                    if up == 5 and "recip_bf" in pend:
                        epi_b()
                    if up < PAIRS:
                        s = up
                        stp = ppool.tile([C, 1024], F32, tag="st", bufs=3)
                        for j in range(2):
                            t = 2 * s + j
                            nc.tensor.matmul(stp[:, 512 * j:512 * j + CHUNK],
                                             k3[:, :, bass.ts(t, 128)],
                                             q3[:, :, sl],
                                             start=True, stop=True, perf_mode=DR)
                        st3 = stp.rearrange("p (b x) -> p b x", b=2)[:, :, 0:CHUNK]
                        ex = spool.tile([C, 2 * CHUNK], FP8, tag="ex", bufs=LAGP + 3)
                        ex3 = ex.rearrange("p (b x) -> p b x", b=2)
                        if act_pat[gidx + s]:
                            nc.scalar.activation(ex3, st3, AF.Exp)
                        else:
                            nc.vector.tensor_scalar(
                                out=ex3.bitcast(I8), in0=st3,
                                scalar1=EXP8_SCALE, scalar2=EXP8_BIAS,
                                op0=mybir.AluOpType.mult,
                                op1=mybir.AluOpType.add)
                        ex_tiles[s] = ex
                    if up >= LAGP:
                        s = up - LAGP
                        ex = ex_tiles.pop(s)
                        ex3 = ex.rearrange("p (b x) -> p b x", b=2)
                        vt3 = vt_sb[:, bass.ds(256 * s, 256)].rearrange(
                            "p (b c) -> p b c", b=2)
                        nc.tensor.matmul(outu[:], vt3, ex3, perf_mode=DR,
                                         start=(s == 0), stop=(s == PAIRS - 1))
                        o3 = ones_db.rearrange("p (b c) -> p b c", b=2)[:, :, 0:1]
                        nc.tensor.matmul(rs[:], o3, ex3, perf_mode=DR,
                                         start=(s == 0), stop=(s == PAIRS - 1))
                pend.update(outu=outu, rs=rs, sl=sl)
            if rep != repeats - 1:
                epi_a()
                epi_b()
                tc.strict_bb_all_engine_barrier()
        if "outu" in pend:
            epi_a()
        if "recip_bf" in pend:
            epi_b()

    nc.compile()
    _BUILD_CACHE[key] = nc
    return nc


def _prep_in_maps(x_q, x_kv, Wq, bq, Wk, bk, Wv, bv, gamma):
    bf16 = ml_dtypes.bfloat16
    f32 = np.float32
    x_q = np.asarray(x_q, f32).reshape(C, N)
    x_kv = np.asarray(x_kv, f32).reshape(C, N)
    Wq = np.asarray(Wq, f32)
    bq = np.asarray(bq, f32)
    Wk = np.asarray(Wk, f32)
    bk = np.asarray(bk, f32)
    Wv = np.asarray(Wv, f32)
    bv = np.asarray(bv, f32)
    gamma = float(np.asarray(gamma, f32).reshape(()))

    scale = 1.0 / np.sqrt(np.float32(RC))
    xkv_b = np.ascontiguousarray(x_kv).astype(bf16)
    wqT = np.ascontiguousarray(Wq.T * scale).astype(bf16)
    wkT = np.ascontiguousarray(Wk.T).astype(bf16)
    wvT = np.ascontiguousarray(Wv.T * gamma).astype(bf16)
    bq_s = np.ascontiguousarray((bq * scale).reshape(RC, 1))
    bk_s = np.ascontiguousarray(bk.reshape(RC, 1))
    resid_bias = (gamma * bv).astype(f32)  # softmax rows sum to 1

    in_maps = []
    for c in range(NCORES):
        xq_slice = np.ascontiguousarray(
            x_q[:, c * NQ:(c + 1) * NQ] + resid_bias[:, None], f32)
        in_maps.append({
            "xq": xq_slice, "xkv": xkv_b,
            "wqT": wqT, "wkT": wkT, "wvT": wvT,
            "bq": bq_s, "bk": bk_s,
        })
    return in_maps


def kernel(x_q, x_kv, Wq, bq, Wk, bk, Wv, bv, gamma):
    nc = build_nc(repeats=1)
    in_maps = _prep_in_maps(x_q, x_kv, Wq, bq, Wk, bk, Wv, bv, gamma)
    res = run_bass_kernel_spmd(nc, in_maps, list(range(NCORES)))
    out = np.concatenate([res.results[c]["y"] for c in range(NCORES)], axis=1)
    return out.reshape(1, C, D, H, W).astype(np.float32)
